# revision 8
# baseline (speedup 1.0000x reference)
"""Trainium2 Bass kernel for nn_KANLayer (B=16384, D=1024, K=8).

Math: the per-feature basis chain collapses algebraically:
    nl[b,i] = sum_k (x[b,i]*W1[i,k] + b1[i,k]) * W2[i,k]
            = x[b,i] * a[i] + c[i],   a = sum_k W1*W2, c = sum_k b1*W2
so the whole layer is ONE dense matmul with a fused diagonal + bias:
    out = x @ (lin_W.T + diag(a)) + (lin_b + c)

Sharding: data-parallel over batch across 8 NeuronCores (2048 rows each);
W_eff (1024x1024) + bias replicated. No collectives needed.

Device kernel (per core): out[2048,1024] = xT.T @ W_eff + bias
  - lhsT (stationary) = xT tile [128 j, 128 b]  (x transposed on host)
  - rhs  (moving)     = W_eff [128 j, 512 i] slices, resident in SBUF
  - psum [128 b, 512 i] f32, accumulated over 8 k-subtiles
  - bias added during PSUM->SBUF eviction on the vector engine
"""

import os
from contextlib import ExitStack

import numpy as np
import ml_dtypes

import concourse.bass as bass
import concourse.tile as tile
from concourse import bacc, mybir
from concourse.bass_utils import run_bass_kernel_spmd

B, D = 16384, 1024
NCORES = 8
BS = B // NCORES  # 2048 batch rows per core
P = 128
KT = D // P   # 8 contraction subtiles
NB = BS // P  # 16 batch tiles per core
NCH = D // 512  # 2 output-feature chunks of 512

# matmul input dtype: bf16 (1 cyc/row on PE, f32 PSUM accumulate)
MM_DT = mybir.dt.bfloat16
MM_NP = ml_dtypes.bfloat16

_CACHE = {}


def _build_nc():
    nc = bacc.Bacc("TRN2", target_bir_lowering=False, debug=False,
                   num_devices=NCORES)
    # xt is host-pre-tiled: xt[bt, p, kt, b] = x[bt*128+b, kt*128+p], so
    # each batch tile is one fully contiguous 256 KB DMA (2 KB/partition).
    xt = nc.dram_tensor("xt", [NB, P, KT, P], MM_DT,
                        kind="ExternalInput").ap()
    w = nc.dram_tensor("w", [D, D], MM_DT, kind="ExternalInput").ap()
    # bias pre-broadcast on host to [128, D] so it loads as one plain
    # contiguous HWDGE DMA.
    bias = nc.dram_tensor("bias", [P, D], mybir.dt.float32,
                          kind="ExternalInput").ap()
    out = nc.dram_tensor("out", [BS, D], mybir.dt.float32,
                         kind="ExternalOutput").ap()

    w_r = w.rearrange("(kt p) n -> kt p n", p=P)
    out_r = out.rearrange("(nb p) n -> nb p n", p=P)

    with tile.TileContext(nc) as tc, ExitStack() as ctx:
        wpool = ctx.enter_context(tc.tile_pool(name="wpool", bufs=1))
        xpool = ctx.enter_context(tc.tile_pool(name="xpool", bufs=8))
        opool = ctx.enter_context(tc.tile_pool(name="opool", bufs=6))
        ppool = ctx.enter_context(tc.tile_pool(name="ppool", bufs=6,
                                               space="PSUM"))

        # PE pre-warm: dummy matmuls on a memset tile keep the PE busy
        # during the input DMA intro so the HAM clock-gate reaches 8/8
        # (2.4 GHz) before the real matmul stream starts.
        warm = wpool.tile([P, 512], MM_DT, tag="warm", name="warm")
        nc.vector.memset(warm, 0.0)
        warm_ps = ppool.tile([P, 512], mybir.dt.float32, tag="ps",
                             name="warm_ps")
        for i in range(8):
            nc.tensor.matmul(warm_ps, lhsT=warm[:, :P], rhs=warm,
                             start=(i == 0), stop=(i == 7))

        # x batch-tile loads on the sync (SP) HWDGE ring. The first two
        # tiles are issued before the weights so the first LDWEIGHTS
        # unblocks as early as possible.
        x_tiles = {}

        def load_x(bt):
            t = xpool.tile([P, KT, P], MM_DT, tag="x", name=f"x_t{bt}")
            nc.sync.dma_start(out=t, in_=xt[bt])
            x_tiles[bt] = t

        load_x(0)
        load_x(1)

        # Resident weights as [128, 512] half tiles; all ch0 halves load
        # before ch1 so the first output chunks are not paced by w DMAs.
        w_t = [[None] * KT for _ in range(NCH)]
        for ch in range(NCH):
            for kt in range(KT):
                wt = wpool.tile([P, 512], MM_DT, tag=f"w{ch}_{kt}",
                                name=f"w_t{ch}_{kt}")
                nc.sync.dma_start(out=wt,
                                  in_=w_r[kt][:, bass.ts(ch, 512)])
                w_t[ch][kt] = wt

        # bias on the scalar (ACT) HWDGE ring, which also carries stores
        bias_t = wpool.tile([P, D], mybir.dt.float32, tag="bias",
                            name="bias_t")
        nc.scalar.dma_start(out=bias_t, in_=bias)

        # compute-group order: start ch0 of the first two batch tiles
        # while the ch1 weight halves are still loading
        groups = [(0, 0), (1, 0), (0, 1), (1, 1)]
        groups += [(bt, ch) for bt in range(2, NB) for ch in range(NCH)]
        next_x = 2

        for bt, ch in groups:
            if next_x < NB:
                load_x(next_x)
                next_x += 1
            x_t = x_tiles[bt]
            psum = ppool.tile([P, 512], mybir.dt.float32, tag="ps",
                              name=f"ps{bt}_{ch}")
            for kt in range(KT):
                nc.tensor.matmul(
                    psum,
                    lhsT=x_t[:, kt, :],
                    rhs=w_t[ch][kt],
                    start=(kt == 0),
                    stop=(kt == KT - 1),
                )
            o_t = opool.tile([P, 512], mybir.dt.float32, tag="o",
                             name=f"o_t{bt}_{ch}")
            nc.vector.tensor_add(o_t, psum, bias_t[:, bass.ts(ch, 512)])
            # store each 512-chunk as soon as it is evicted, on the
            # scalar HWDGE ring (keeps the sync ring free for loads)
            nc.scalar.dma_start(out=out_r[bt][:, bass.ts(ch, 512)],
                                in_=o_t)

    nc.compile()
    return nc


def _get_nc():
    if "nc" not in _CACHE:
        _CACHE["nc"] = _build_nc()
    return _CACHE["nc"]


def _prep_inputs(x, lin_W, lin_b, W1, b1, W2):
    """Host-side prep: fold the per-feature basis chain into the matmul."""
    x = np.asarray(x, dtype=np.float32)
    lin_W = np.asarray(lin_W, dtype=np.float32)
    a = np.sum(np.asarray(W1, np.float32) * np.asarray(W2, np.float32),
               axis=1)
    c = np.sum(np.asarray(b1, np.float32) * np.asarray(W2, np.float32),
               axis=1)
    W_eff = np.ascontiguousarray(lin_W.T)
    idx = np.arange(D)
    W_eff[idx, idx] += a
    bias = (np.asarray(lin_b, np.float32) + c).astype(np.float32)
    bias2d = np.ascontiguousarray(np.broadcast_to(bias, (P, D)))

    # Pre-tile x per core: xt[core][bt, p, kt, b] = x_shard[bt*128+b,
    # kt*128+p] — contiguous 256 KB per batch tile on device.
    xb = x.astype(MM_NP).reshape(NCORES, NB, P, KT, P)
    xt = np.ascontiguousarray(xb.transpose(0, 1, 4, 3, 2))
    w_dev = W_eff.astype(MM_NP)
    return xt, w_dev, bias2d


def kernel(x, lin_W, lin_b, W1, b1, W2):
    xt, w_dev, bias2d = _prep_inputs(x, lin_W, lin_b, W1, b1, W2)
    in_maps = [
        {"xt": xt[i], "w": w_dev, "bias": bias2d}
        for i in range(NCORES)
    ]
    nc = _get_nc()
    res = run_bass_kernel_spmd(nc, in_maps, core_ids=list(range(NCORES)))
    out = np.concatenate([r["out"] for r in res.results], axis=0)
    return np.ascontiguousarray(out.astype(np.float32))


# revision 10
# speedup vs baseline: 1.0587x; 1.0587x over previous
"""Trainium2 Bass kernel for nn_KANLayer (B=16384, D=1024, K=8).

Math: the per-feature basis chain collapses algebraically:
    nl[b,i] = sum_k (x[b,i]*W1[i,k] + b1[i,k]) * W2[i,k]
            = x[b,i] * a[i] + c[i],   a = sum_k W1*W2, c = sum_k b1*W2
so the whole layer is ONE dense matmul with a fused diagonal + bias:
    out = x @ (lin_W.T + diag(a)) + (lin_b + c)

Sharding: data-parallel over batch across 8 NeuronCores (2048 rows each);
W_eff (1024x1024) + bias replicated. No collectives needed.

Device kernel (per core): out[2048,1024] = xT.T @ W_eff + bias
  - lhsT (stationary) = xT tile [128 j, 128 b]  (x transposed on host)
  - rhs  (moving)     = W_eff [128 j, 512 i] slices, resident in SBUF
  - psum [128 b, 512 i] f32, accumulated over 8 k-subtiles
  - bias added during PSUM->SBUF eviction on the vector engine
"""

import os
from contextlib import ExitStack

import numpy as np
import ml_dtypes

import concourse.bass as bass
import concourse.tile as tile
from concourse import bacc, mybir
from concourse.bass_utils import run_bass_kernel_spmd

B, D = 16384, 1024
NCORES = 8
BS = B // NCORES  # 2048 batch rows per core
P = 128
KT = D // P   # 8 contraction subtiles
NB = BS // P  # 16 batch tiles per core
NCH = D // 512  # 2 output-feature chunks of 512

# matmul input dtype: bf16 (1 cyc/row on PE, f32 PSUM accumulate)
MM_DT = mybir.dt.bfloat16
MM_NP = ml_dtypes.bfloat16

_CACHE = {}


def _build_nc():
    nc = bacc.Bacc("TRN2", target_bir_lowering=False, debug=False,
                   num_devices=NCORES)
    # xt is host-pre-tiled: xt[bt, p, kt, b] = x[bt*128+b, kt*128+p], so
    # each batch tile is one fully contiguous 256 KB DMA (2 KB/partition).
    xt = nc.dram_tensor("xt", [NB, P, KT, P], MM_DT,
                        kind="ExternalInput").ap()
    w = nc.dram_tensor("w", [D, D], MM_DT, kind="ExternalInput").ap()
    # bias pre-broadcast on host to [128, D] so it loads as one plain
    # contiguous HWDGE DMA.
    bias = nc.dram_tensor("bias", [P, D], mybir.dt.float32,
                          kind="ExternalInput").ap()
    out = nc.dram_tensor("out", [BS, D], mybir.dt.float32,
                         kind="ExternalOutput").ap()

    w_r = w.rearrange("(kt p) n -> kt p n", p=P)
    out_r = out.rearrange("(nb p) n -> nb p n", p=P)

    with tile.TileContext(nc) as tc, ExitStack() as ctx:
        wpool = ctx.enter_context(tc.tile_pool(name="wpool", bufs=1))
        xpool = ctx.enter_context(tc.tile_pool(name="xpool", bufs=4))
        opool = ctx.enter_context(tc.tile_pool(name="opool", bufs=4))
        ppool = ctx.enter_context(tc.tile_pool(name="ppool", bufs=6,
                                               space="PSUM"))

        # PE pre-warm: dummy matmuls on a memset tile keep the PE busy
        # during the input DMA intro so the HAM clock-gate reaches 8/8
        # (2.4 GHz) before (or soon after) the real matmul stream starts.
        warm = wpool.tile([P, 512], MM_DT, tag="warm", name="warm")
        nc.vector.memset(warm, 0.0)
        warm_ps = ppool.tile([P, 512], mybir.dt.float32, tag="ps",
                             name="warm_ps")
        for i in range(12):
            nc.tensor.matmul(warm_ps, lhsT=warm[:, :P], rhs=warm,
                             start=(i == 0), stop=(i == 11))

        # x loads in 2-batch-tile pairs (contiguous 512 KB DMAs) on the
        # sync (SP) HWDGE ring; first pair issued before the weights.
        x_tiles = {}

        def load_x_pair(pair):
            t = xpool.tile([P, 2, KT, P], MM_DT, tag="x",
                           name=f"x_t{pair}")
            nc.sync.dma_start(
                out=t, in_=xt[2 * pair:2 * pair + 2].rearrange(
                    "n p k b -> p n k b"))
            x_tiles[pair] = t

        load_x_pair(0)

        # Resident weights: 8 tiles [128, 1024], one per k-subtile
        w_t = []
        for kt in range(KT):
            wt = wpool.tile([P, D], MM_DT, tag=f"w{kt}", name=f"w_t{kt}")
            nc.sync.dma_start(out=wt, in_=w_r[kt])
            w_t.append(wt)

        # bias on the scalar (ACT) HWDGE ring, which also carries stores
        bias_t = wpool.tile([P, D], mybir.dt.float32, tag="bias",
                            name="bias_t")
        nc.scalar.dma_start(out=bias_t, in_=bias)

        for bt in range(NB):
            pair, sub = divmod(bt, 2)
            if sub == 0 and pair + 1 < NB // 2:
                load_x_pair(pair + 1)
            x_t = x_tiles[pair]
            o_t = opool.tile([P, D], mybir.dt.float32, tag="o",
                             name=f"o_t{bt}")
            for ch in range(NCH):
                psum = ppool.tile([P, 512], mybir.dt.float32, tag="ps",
                                  name=f"ps{bt}_{ch}")
                for kt in range(KT):
                    nc.tensor.matmul(
                        psum,
                        lhsT=x_t[:, sub, kt, :],
                        rhs=w_t[kt][:, bass.ts(ch, 512)],
                        start=(kt == 0),
                        stop=(kt == KT - 1),
                    )
                nc.vector.tensor_add(o_t[:, bass.ts(ch, 512)], psum,
                                     bias_t[:, bass.ts(ch, 512)])
            # one 512 KB store per batch tile on the scalar HWDGE ring
            # (keeps the sync ring free for loads)
            nc.scalar.dma_start(out=out_r[bt], in_=o_t)

    nc.compile()
    return nc


def _get_nc():
    if "nc" not in _CACHE:
        _CACHE["nc"] = _build_nc()
    return _CACHE["nc"]


def _prep_inputs(x, lin_W, lin_b, W1, b1, W2):
    """Host-side prep: fold the per-feature basis chain into the matmul."""
    x = np.asarray(x, dtype=np.float32)
    lin_W = np.asarray(lin_W, dtype=np.float32)
    a = np.sum(np.asarray(W1, np.float32) * np.asarray(W2, np.float32),
               axis=1)
    c = np.sum(np.asarray(b1, np.float32) * np.asarray(W2, np.float32),
               axis=1)
    W_eff = np.ascontiguousarray(lin_W.T)
    idx = np.arange(D)
    W_eff[idx, idx] += a
    bias = (np.asarray(lin_b, np.float32) + c).astype(np.float32)
    bias2d = np.ascontiguousarray(np.broadcast_to(bias, (P, D)))

    # Pre-tile x per core: xt[core][bt, p, kt, b] = x_shard[bt*128+b,
    # kt*128+p] — contiguous 256 KB per batch tile on device.
    xb = x.astype(MM_NP).reshape(NCORES, NB, P, KT, P)
    xt = np.ascontiguousarray(xb.transpose(0, 1, 4, 3, 2))
    w_dev = W_eff.astype(MM_NP)
    return xt, w_dev, bias2d


def kernel(x, lin_W, lin_b, W1, b1, W2):
    xt, w_dev, bias2d = _prep_inputs(x, lin_W, lin_b, W1, b1, W2)
    in_maps = [
        {"xt": xt[i], "w": w_dev, "bias": bias2d}
        for i in range(NCORES)
    ]
    nc = _get_nc()
    res = run_bass_kernel_spmd(nc, in_maps, core_ids=list(range(NCORES)))
    out = np.concatenate([r["out"] for r in res.results], axis=0)
    return np.ascontiguousarray(out.astype(np.float32))
